# revision 25
# baseline (speedup 1.0000x reference)
"""CenterLoss kernel for 8 Trainium2 NeuronCores (Bass, raw engine blocks).

Strategy (matches the row-sharded centers-bank hint):
  * centers [100000, 256] is sharded row-wise: core k owns classes
    [k*12500, (k+1)*12500).  All occurrences of a class land on exactly
    one core, so per-class counts and the segment-scatter are core-local.
  * (feature, target) pairs are routed to the owning core on the host and
    laid out in a fixed 1664-position grid of 13 tiles x 128 slots:
       tiles 0..9    single-occurrence classes of row band b
                     (band b = shard rows [1250b, 1250(b+1))), ascending
       tile 10       "overflow": all classes with >=2 occurrences, plus
                     any band slots beyond 128 (first occurrences)
       tile 11       2nd occurrences, aligned to tile-10 slots
       tile 12       3rd occurrences, aligned to tile-10 slots
    Band alignment lets the scatter of tile b start as soon as the bulk
    copy of row band b has landed, hiding the scatter under the copy.
  * Pad positions gather shard row 0 and carry shard-row-0 as their
    "feature", so their diff is exactly 0 (no loss / delta pollution).
    Pad scatter slots point at row 12500: out of bounds -> dropped via
    the DMA bounds check.
  * Device per core: indirect-gather the 1664 center rows (one 128-row
    tile per indirect DMA - hardware consumes one index per partition),
    diff against the routed features, square+reduce for the loss
    partial, fold 2nd/3rd occurrences into their tile-10 slots, fused
    (diff * -1/(count+eps)) + center update, 10-chunk DRAM->DRAM copy of
    the shard into the output with per-chunk semaphores, band scatters
    chasing the copy chunks, overflow scatter last.
  * Host: concatenate the 8 output shards, sum the loss partials.

The program is written as raw engine blocks with explicit semaphores
(the neuronxcc codegen used on this path allows at most one semaphore
wait per instruction, which Tile's auto-sync does not respect; raw
wait_ge instructions carry exactly one wait each).
"""

import numpy as np

from concourse import bass, mybir
from concourse import bass_utils
from concourse.bass import IndirectOffsetOnAxis

ALPHA = 1.0
EPS = 1e-6

NUM_CLASSES = 100000
FEAT_DIM = 256
BATCH = 8192
M = 8                       # cores
CPC = NUM_CLASSES // M      # classes per core = 12500

N_BAND = 10                 # scatter row bands
BAND_ROWS = CPC // N_BAND   # 1250 rows per band
COPY_ROWS = 1024            # bulk-copy chunk rows (64 KB per DMA engine)
N_CHUNK = (CPC + COPY_ROWS - 1) // COPY_ROWS   # 13 chunks (last is 212 rows)
U_BAND = N_BAND * 128       # 1280 band slots
OV_CAP = 128                # overflow slots (duplicates + band spill)
N_UT = N_BAND + 1           # 11 scatter tiles (10 bands + overflow)
N_T = N_UT + 2              # 13 grid tiles (+ 2nd/3rd occurrence tiles)
GRID = N_T * 128            # 1664 grid positions
N_SLOTS = N_UT * 128        # 1408 scatter slots

_PROGRAM = None


def _build_program():
    f32 = mybir.dt.float32
    i32 = mybir.dt.int32
    D = FEAT_DIM

    nc = bass.Bass("TRN2", num_devices=M, num_swdge_queues=2)

    cin = nc.dram_tensor("cin", [CPC, D], f32, kind="ExternalInput").ap()
    feats = nc.dram_tensor("feats", [128, N_T * D], f32, kind="ExternalInput").ap()
    # gidx (cols 0:N_T) and sidx (cols N_T:N_T+N_UT) packed in one tensor
    meta = nc.dram_tensor("meta", [128, N_T + N_UT], i32, kind="ExternalInput").ap()
    nscale = nc.dram_tensor("nscale", [128, N_UT], f32, kind="ExternalInput").ap()
    cout = nc.dram_tensor("cout", [CPC, D], f32, kind="ExternalOutput").ap()
    lossv = nc.dram_tensor("lossv", [128, 1], f32, kind="ExternalOutput").ap()

    meta_t = nc.alloc_sbuf_tensor("meta_t", [128, N_T + N_UT], i32).ap()
    nscale_t = nc.alloc_sbuf_tensor("nscale_t", [128, N_UT], f32).ap()
    F = nc.alloc_sbuf_tensor("F", [128, N_T * D], f32).ap()
    G = nc.alloc_sbuf_tensor("G", [128, N_T * D], f32).ap()
    DF = nc.alloc_sbuf_tensor("DF", [128, N_T * D], f32).ap()
    loss_col = nc.alloc_sbuf_tensor("loss_col", [128, 1], f32).ap()

    ov = N_BAND * D          # overflow tile (10) columns
    r1 = (N_BAND + 1) * D    # 2nd-occurrence tile (11) columns
    r2 = (N_BAND + 2) * D    # 3rd-occurrence tile (12) columns

    from contextlib import ExitStack
    with ExitStack() as _stack:
        block = _stack.enter_context(nc.Block())
        sw = _stack.enter_context(nc.semaphore("sw"))  # SWDGE completions
        mt = _stack.enter_context(nc.semaphore("mt"))  # meta load completion
        al = _stack.enter_context(nc.semaphore("al"))  # loss store completion
        dv = _stack.enter_context(nc.semaphore("dv"))  # DVE milestones
        # per-chunk bulk-copy completions
        hw = [_stack.enter_context(nc.semaphore(f"hw{c}")) for c in range(N_CHUNK)]
        gz = _stack.enter_context(nc.semaphore("gz"))  # G zeroed
        @block.gpsimd
        def _(eng):
            # F first on the SWDGE ring (contiguous, fat descriptors),
            # then nscale; the ring drains FIFO per engine so both finish
            # before the index-bound gathers.
            eng.dma_start(out=F[:], in_=feats[:]).then_inc(sw, 16)
            eng.dma_start(out=nscale_t[:], in_=nscale[:]).then_inc(sw, 16)
            eng.wait_ge(mt, 16)          # indices are in SBUF (via HWDGE)
            eng.wait_ge(gz, 1)           # G tile zeroed (pad slots stay 0)
            # indirect DMA consumes ONE index per partition on hardware, so
            # gather/scatter go one 128-row tile at a time.  Pad positions
            # carry index 12500 -> dropped by the bounds check, saving their
            # descriptors and HBM traffic.
            for t in range(N_T):
                ins = eng.indirect_dma_start(
                    out=G[:, t * D:(t + 1) * D],
                    out_offset=None,
                    in_=cin[:],
                    in_offset=IndirectOffsetOnAxis(ap=meta_t[:, t:t + 1], axis=0),
                    bounds_check=CPC - 1,
                    oob_is_err=False,
                )
                if t % 2 == 1:
                    ins.ins.queue = "qPoolDynamic1"
                ins.then_inc(sw, 16)
            eng.wait_ge(dv, 2)           # updated rows ready in G
            # band scatters chase the copy chunks; pad slots point at row
            # 12500 -> out of bounds -> dropped
            for t in range(N_UT):
                if t < N_BAND:
                    # last copy chunk covering band t; HWDGE FIFO per issuing
                    # engine makes "chunk c done on all 16 engines" imply all
                    # earlier chunks are done too
                    c_hi = ((t + 1) * BAND_ROWS - 1) // COPY_ROWS
                    eng.wait_ge(hw[c_hi], 16)
                # overflow tile: all hw waits already observed in order
                ins = eng.indirect_dma_start(
                    out=cout[:],
                    out_offset=IndirectOffsetOnAxis(
                        ap=meta_t[:, N_T + t:N_T + t + 1], axis=0
                    ),
                    in_=G[:, t * D:(t + 1) * D],
                    in_offset=None,
                    bounds_check=CPC - 1,
                    oob_is_err=False,
                )
                if t % 2 == 1:
                    ins.ins.queue = "qPoolDynamic1"  # parallel desc-gen
                ins.then_inc(sw, 16)
            eng.wait_ge(sw, (2 + N_T + N_UT) * 16)   # all SWDGE DMAs landed

        @block.vector
        def _(eng):
            eng.memset(G[:], 0.0)
            ins = eng.sem_inc(gz, 1)
            eng.wait_ge(sw, (2 + N_T) * 16)   # nscale, feats, gathers done
            # diff = center - feature (pads are exactly 0)
            eng.tensor_tensor(
                out=DF[:], in0=G[:], in1=F[:], op=mybir.AluOpType.subtract
            )
            # loss partial: per-partition sum of diff^2 (squares scribble
            # over F, which is dead after the subtract)
            eng.tensor_tensor(
                out=F[:], in0=DF[:], in1=DF[:], op=mybir.AluOpType.mult
            )
            eng.tensor_reduce(
                out=loss_col[:], in_=F[:],
                axis=mybir.AxisListType.X, op=mybir.AluOpType.add,
            ).then_inc(dv, 1)
            # fold 2nd/3rd occurrences into their overflow slots (tile 10)
            eng.tensor_tensor(
                out=DF[:, ov:ov + D], in0=DF[:, ov:ov + D],
                in1=DF[:, r1:r1 + D], op=mybir.AluOpType.add,
            )
            eng.tensor_tensor(
                out=DF[:, ov:ov + D], in0=DF[:, ov:ov + D],
                in1=DF[:, r2:r2 + D], op=mybir.AluOpType.add,
            )
            # updated rows: G = (DF * -1/(count+eps)) + G per slot tile
            for t in range(N_UT):
                sl = slice(t * D, (t + 1) * D)
                ins = eng.scalar_tensor_tensor(
                    out=G[:, sl],
                    in0=DF[:, sl],
                    scalar=nscale_t[:, t:t + 1],
                    in1=G[:, sl],
                    op0=mybir.AluOpType.mult,
                    op1=mybir.AluOpType.add,
                )
            ins.then_inc(dv, 1)

        @block.scalar
        def _(eng):
            eng.wait_ge(dv, 1)
            eng.dma_start(out=lossv[:], in_=loss_col[:]).then_inc(al, 16)
            eng.wait_ge(al, 16)

        @block.sync
        def _(eng):
            # index load via the otherwise-idle HWDGE path: lands fast and
            # lets the gathers start while the SWDGE ring drains F
            eng.dma_start(out=meta_t[:], in_=meta[:]).then_inc(mt, 16)
            # stage the bulk copy: a few chunks ride along with the gather
            # phase (the DMA engines are mostly idle while the Q7 generates
            # gather descriptors), the rest wait so the copy's fat packets
            # don't starve the 1 KB gather packets
            EARLY = 6
            eng.wait_ge(mt, 16)
            for c in range(EARLY):
                r0 = c * COPY_ROWS
                eng.dma_start(
                    out=cout[r0:r0 + COPY_ROWS, :],
                    in_=cin[r0:r0 + COPY_ROWS, :],
                ).then_inc(hw[c], 16)
            eng.wait_ge(sw, (2 + N_T) * 16)
            for c in range(EARLY, N_CHUNK):
                r0 = c * COPY_ROWS
                r1 = min(r0 + COPY_ROWS, CPC)
                eng.dma_start(
                    out=cout[r0:r1, :],
                    in_=cin[r0:r1, :],
                ).then_inc(hw[c], 16)

    return nc


def _route(features, target, centers):
    """Build the 8 per-core input maps."""
    feats_np = np.ascontiguousarray(np.asarray(features, dtype=np.float32))
    tgt = np.asarray(target).astype(np.int64)
    centers_np = np.asarray(centers, dtype=np.float32)
    if not centers_np.flags.c_contiguous:
        centers_np = np.ascontiguousarray(centers_np)

    in_maps = []
    for k in range(M):
        base = k * CPC
        sel = np.nonzero((tgt >= base) & (tgt < base + CPC))[0]
        loc = (tgt[sel] - base).astype(np.int64)

        uniq, inv, cnt = np.unique(loc, return_inverse=True, return_counts=True)
        nu = len(uniq)
        if cnt.max(initial=0) > 3:
            raise ValueError(f"core {k}: class multiplicity {cnt.max()} > 3")

        # slot assignment: band slots for singles, overflow for the rest
        slot_of_uniq = np.full(nu, -1, dtype=np.int64)
        is_dup = cnt >= 2
        overflow = np.nonzero(is_dup)[0].tolist()
        for b in range(N_BAND):
            idxs = np.nonzero((~is_dup) & (uniq // BAND_ROWS == b))[0]
            take = idxs[:128]
            slot_of_uniq[take] = 128 * b + np.arange(len(take))
            overflow.extend(idxs[128:].tolist())
        overflow = np.array(sorted(overflow), dtype=np.int64)
        if len(overflow) > OV_CAP:
            raise ValueError(f"core {k}: overflow {len(overflow)} > {OV_CAP}")
        slot_of_uniq[overflow] = U_BAND + np.arange(len(overflow))

        # occurrence rank of each pair within its class (batch order preserved)
        o = np.argsort(inv, kind="stable")
        grp = inv[o]
        starts = np.r_[True, grp[1:] != grp[:-1]]
        pos_in_sorted = np.arange(len(inv))
        run_start = np.maximum.accumulate(np.where(starts, pos_in_sorted, 0))
        occ = np.empty(len(inv), dtype=np.int64)
        occ[o] = pos_in_sorted - run_start

        slot = slot_of_uniq[inv]
        if not (slot[occ >= 1] >= U_BAND).all():
            raise ValueError(f"core {k}: duplicate class outside overflow")
        gridpos = np.where(
            occ == 0, slot,
            np.where(occ == 1,
                     (N_UT * 128) + (slot - U_BAND),
                     ((N_UT + 1) * 128) + (slot - U_BAND)),
        )

        shard = centers_np[base:base + CPC]       # contiguous view, no copy
        row0 = shard[0]

        fg = np.zeros((128, N_T, FEAT_DIM), dtype=np.float32)  # pads: 0
        fg[gridpos % 128, gridpos // 128] = feats_np[sel]
        fg = fg.reshape(128, N_T * FEAT_DIM)

        gi = np.full(GRID, CPC, dtype=np.int32)   # pads dropped (OOB)
        gi[gridpos] = loc.astype(np.int32)

        si = np.full(N_SLOTS, CPC, dtype=np.int32)  # pads scatter out of bounds
        si[slot_of_uniq] = uniq.astype(np.int32)

        ns = np.zeros(N_SLOTS, dtype=np.float32)
        ns[slot_of_uniq] = -(
            np.float32(ALPHA) / (cnt.astype(np.float32) + np.float32(EPS))
        )

        meta = np.concatenate(
            [gi.reshape(N_T, 128).T, si.reshape(N_UT, 128).T], axis=1
        )
        in_maps.append({
            "cin": shard,
            "feats": fg,
            "meta": np.ascontiguousarray(meta),
            "nscale": np.ascontiguousarray(ns.reshape(N_UT, 128).T),
        })
    return in_maps


def kernel(features, target, centers):
    global _PROGRAM
    if _PROGRAM is None:
        _PROGRAM = _build_program()

    in_maps = _route(features, target, centers)
    res = bass_utils.run_bass_kernel_spmd(
        _PROGRAM, in_maps, core_ids=list(range(M))
    ).results

    new_centers = np.empty((NUM_CLASSES, FEAT_DIM), dtype=np.float32)
    loss_sum = 0.0
    for k in range(M):
        new_centers[k * CPC:(k + 1) * CPC] = res[k]["cout"]
        loss_sum += float(np.sum(res[k]["lossv"], dtype=np.float64))

    center_loss = np.float32(loss_sum / (BATCH * FEAT_DIM))
    return center_loss, new_centers


# revision 26
# speedup vs baseline: 1.0842x; 1.0842x over previous
"""CenterLoss kernel for 8 Trainium2 NeuronCores (Bass, raw engine blocks).

Strategy (matches the row-sharded centers-bank hint):
  * centers [100000, 256] is sharded row-wise: core k owns classes
    [k*12500, (k+1)*12500).  All occurrences of a class land on exactly
    one core, so per-class counts and the segment-scatter are core-local.
  * (feature, target) pairs are routed to the owning core on the host and
    laid out in a fixed 1664-position grid of 13 tiles x 128 slots:
       tiles 0..9    single-occurrence classes of row band b
                     (band b = shard rows [1250b, 1250(b+1))), ascending
       tile 10       "overflow": all classes with >=2 occurrences, plus
                     any band slots beyond 128 (first occurrences)
       tile 11       2nd occurrences, aligned to tile-10 slots
       tile 12       3rd occurrences, aligned to tile-10 slots
    Band alignment lets the scatter of tile b start as soon as the bulk
    copy of row band b has landed, hiding the scatter under the copy.
  * Pad positions gather shard row 0 and carry shard-row-0 as their
    "feature", so their diff is exactly 0 (no loss / delta pollution).
    Pad scatter slots point at row 12500: out of bounds -> dropped via
    the DMA bounds check.
  * Device per core: indirect-gather the 1664 center rows (one 128-row
    tile per indirect DMA - hardware consumes one index per partition),
    diff against the routed features, square+reduce for the loss
    partial, fold 2nd/3rd occurrences into their tile-10 slots, fused
    (diff * -1/(count+eps)) + center update, 10-chunk DRAM->DRAM copy of
    the shard into the output with per-chunk semaphores, band scatters
    chasing the copy chunks, overflow scatter last.
  * Host: concatenate the 8 output shards, sum the loss partials.

The program is written as raw engine blocks with explicit semaphores
(the neuronxcc codegen used on this path allows at most one semaphore
wait per instruction, which Tile's auto-sync does not respect; raw
wait_ge instructions carry exactly one wait each).
"""

import numpy as np

from concourse import bass, mybir
from concourse import bass_utils
from concourse.bass import IndirectOffsetOnAxis

ALPHA = 1.0
EPS = 1e-6

NUM_CLASSES = 100000
FEAT_DIM = 256
BATCH = 8192
M = 8                       # cores
CPC = NUM_CLASSES // M      # classes per core = 12500

N_BAND = 10                 # scatter row bands
BAND_ROWS = CPC // N_BAND   # 1250 rows per band
COPY_ROWS = 1024            # bulk-copy chunk rows (64 KB per DMA engine)
N_CHUNK = (CPC + COPY_ROWS - 1) // COPY_ROWS   # 13 chunks (last is 212 rows)
U_BAND = N_BAND * 128       # 1280 band slots
OV_CAP = 128                # overflow slots (duplicates + band spill)
N_UT = N_BAND + 1           # 11 scatter tiles (10 bands + overflow)
N_T = N_UT + 2              # 13 grid tiles (+ 2nd/3rd occurrence tiles)
GRID = N_T * 128            # 1664 grid positions
N_SLOTS = N_UT * 128        # 1408 scatter slots

_PROGRAM = None


def _build_program():
    f32 = mybir.dt.float32
    i32 = mybir.dt.int32
    D = FEAT_DIM

    nc = bass.Bass("TRN2", num_devices=M, num_swdge_queues=2)

    cin = nc.dram_tensor("cin", [CPC, D], f32, kind="ExternalInput").ap()
    feats = nc.dram_tensor("feats", [128, N_T * D], f32, kind="ExternalInput").ap()
    # gidx (cols 0:N_T) and sidx (cols N_T:N_T+N_UT) packed in one tensor
    meta = nc.dram_tensor("meta", [128, N_T + N_UT], i32, kind="ExternalInput").ap()
    nscale = nc.dram_tensor("nscale", [128, N_UT], f32, kind="ExternalInput").ap()
    cout = nc.dram_tensor("cout", [CPC, D], f32, kind="ExternalOutput").ap()
    lossv = nc.dram_tensor("lossv", [128, 1], f32, kind="ExternalOutput").ap()

    meta_t = nc.alloc_sbuf_tensor("meta_t", [128, N_T + N_UT], i32).ap()
    nscale_t = nc.alloc_sbuf_tensor("nscale_t", [128, N_UT], f32).ap()
    F = nc.alloc_sbuf_tensor("F", [128, N_T * D], f32).ap()
    G = nc.alloc_sbuf_tensor("G", [128, N_T * D], f32).ap()
    DF = nc.alloc_sbuf_tensor("DF", [128, N_T * D], f32).ap()
    loss_col = nc.alloc_sbuf_tensor("loss_col", [128, 1], f32).ap()
    OV = nc.alloc_sbuf_tensor("OV", [128, FEAT_DIM], f32).ap()

    ov = N_BAND * D          # overflow tile (10) columns
    r1 = (N_BAND + 1) * D    # 2nd-occurrence tile (11) columns
    r2 = (N_BAND + 2) * D    # 3rd-occurrence tile (12) columns

    from contextlib import ExitStack
    with ExitStack() as _stack:
        block = _stack.enter_context(nc.Block())
        sw = _stack.enter_context(nc.semaphore("sw"))  # SWDGE completions
        mt = _stack.enter_context(nc.semaphore("mt"))  # meta load completion
        al = _stack.enter_context(nc.semaphore("al"))  # loss store completion
        dv = _stack.enter_context(nc.semaphore("dv"))  # DVE milestones
        # per-chunk bulk-copy completions
        hw = [_stack.enter_context(nc.semaphore(f"hw{c}")) for c in range(N_CHUNK)]
        gz = _stack.enter_context(nc.semaphore("gz"))  # G zeroed
        @block.gpsimd
        def _(eng):
            # F first on the SWDGE ring (contiguous, fat descriptors),
            # then nscale; the ring drains FIFO per engine so both finish
            # before the index-bound gathers.
            eng.dma_start(out=F[:], in_=feats[:]).then_inc(sw, 16)
            eng.dma_start(out=nscale_t[:], in_=nscale[:]).then_inc(sw, 16)
            eng.wait_ge(mt, 16)          # indices are in SBUF (via HWDGE)
            eng.wait_ge(gz, 1)           # G tile zeroed (pad slots stay 0)
            # indirect DMA consumes ONE index per partition on hardware, so
            # gather/scatter go one 128-row tile at a time.  Pad positions
            # carry index 12500 -> dropped by the bounds check, saving their
            # descriptors and HBM traffic.
            for t in range(N_T):
                ins = eng.indirect_dma_start(
                    out=G[:, t * D:(t + 1) * D],
                    out_offset=None,
                    in_=cin[:],
                    in_offset=IndirectOffsetOnAxis(ap=meta_t[:, t:t + 1], axis=0),
                    bounds_check=CPC - 1,
                    oob_is_err=False,
                )
                if t % 2 == 1:
                    ins.ins.queue = "qPoolDynamic1"
                ins.then_inc(sw, 16)
            eng.wait_ge(dv, 1)           # updated rows ready in G
            # band scatters chase the copy chunks; pad slots point at row
            # 12500 -> out of bounds -> dropped
            for t in range(N_UT):
                if t < N_BAND:
                    # last copy chunk covering band t; HWDGE FIFO per issuing
                    # engine makes "chunk c done on all 16 engines" imply all
                    # earlier chunks are done too
                    c_hi = ((t + 1) * BAND_ROWS - 1) // COPY_ROWS
                    eng.wait_ge(hw[c_hi], 16)
                # overflow tile: all hw waits already observed in order
                ins = eng.indirect_dma_start(
                    out=cout[:],
                    out_offset=IndirectOffsetOnAxis(
                        ap=meta_t[:, N_T + t:N_T + t + 1], axis=0
                    ),
                    in_=G[:, t * D:(t + 1) * D],
                    in_offset=None,
                    bounds_check=CPC - 1,
                    oob_is_err=False,
                )
                if t % 2 == 1:
                    ins.ins.queue = "qPoolDynamic1"  # parallel desc-gen
                ins.then_inc(sw, 16)
            eng.wait_ge(sw, (2 + N_T + N_UT) * 16)   # all SWDGE DMAs landed

        @block.vector
        def _(eng):
            eng.memset(G[:], 0.0)
            ins = eng.sem_inc(gz, 1)
            eng.wait_ge(sw, (2 + N_T) * 16)   # nscale, feats, gathers done
            # diff = center - feature (pads are exactly 0)
            eng.tensor_tensor(
                out=DF[:], in0=G[:], in1=F[:], op=mybir.AluOpType.subtract
            )
            # scatter-gating path first: fold 2nd/3rd occurrences into a
            # separate overflow-delta buffer (DF itself must stay pristine
            # for the loss), then produce the updated rows in G
            eng.tensor_tensor(
                out=OV[:], in0=DF[:, ov:ov + D],
                in1=DF[:, r1:r1 + D], op=mybir.AluOpType.add,
            )
            eng.tensor_tensor(
                out=OV[:], in0=OV[:],
                in1=DF[:, r2:r2 + D], op=mybir.AluOpType.add,
            )
            # updated rows: G = (delta * -1/(count+eps)) + G per slot tile
            for t in range(N_UT):
                sl = slice(t * D, (t + 1) * D)
                src_ap = OV[:] if t == N_BAND else DF[:, sl]
                ins = eng.scalar_tensor_tensor(
                    out=G[:, sl],
                    in0=src_ap,
                    scalar=nscale_t[:, t:t + 1],
                    in1=G[:, sl],
                    op0=mybir.AluOpType.mult,
                    op1=mybir.AluOpType.add,
                )
            ins.then_inc(dv, 1)              # scatters may go
            # loss partial: per-partition sum of diff^2 (squares scribble
            # over F, which is dead after the subtract)
            eng.tensor_tensor(
                out=F[:], in0=DF[:], in1=DF[:], op=mybir.AluOpType.mult
            )
            eng.tensor_reduce(
                out=loss_col[:], in_=F[:],
                axis=mybir.AxisListType.X, op=mybir.AluOpType.add,
            ).then_inc(dv, 1)

        @block.scalar
        def _(eng):
            eng.wait_ge(dv, 2)
            eng.dma_start(out=lossv[:], in_=loss_col[:]).then_inc(al, 16)
            eng.wait_ge(al, 16)

        @block.sync
        def _(eng):
            # index load via the otherwise-idle HWDGE path: lands fast and
            # lets the gathers start while the SWDGE ring drains F
            eng.dma_start(out=meta_t[:], in_=meta[:]).then_inc(mt, 16)
            # stage the bulk copy: a few chunks ride along with the gather
            # phase (the DMA engines are mostly idle while the Q7 generates
            # gather descriptors), the rest wait so the copy's fat packets
            # don't starve the 1 KB gather packets
            EARLY = 6
            eng.wait_ge(mt, 16)
            for c in range(EARLY):
                r0 = c * COPY_ROWS
                eng.dma_start(
                    out=cout[r0:r0 + COPY_ROWS, :],
                    in_=cin[r0:r0 + COPY_ROWS, :],
                ).then_inc(hw[c], 16)
            eng.wait_ge(sw, (2 + N_T) * 16)
            for c in range(EARLY, N_CHUNK):
                r0 = c * COPY_ROWS
                r1 = min(r0 + COPY_ROWS, CPC)
                eng.dma_start(
                    out=cout[r0:r1, :],
                    in_=cin[r0:r1, :],
                ).then_inc(hw[c], 16)

    return nc


def _route(features, target, centers):
    """Build the 8 per-core input maps."""
    feats_np = np.ascontiguousarray(np.asarray(features, dtype=np.float32))
    tgt = np.asarray(target).astype(np.int64)
    centers_np = np.asarray(centers, dtype=np.float32)
    if not centers_np.flags.c_contiguous:
        centers_np = np.ascontiguousarray(centers_np)

    in_maps = []
    for k in range(M):
        base = k * CPC
        sel = np.nonzero((tgt >= base) & (tgt < base + CPC))[0]
        loc = (tgt[sel] - base).astype(np.int64)

        uniq, inv, cnt = np.unique(loc, return_inverse=True, return_counts=True)
        nu = len(uniq)
        if cnt.max(initial=0) > 3:
            raise ValueError(f"core {k}: class multiplicity {cnt.max()} > 3")

        # slot assignment: band slots for singles, overflow for the rest
        slot_of_uniq = np.full(nu, -1, dtype=np.int64)
        is_dup = cnt >= 2
        overflow = np.nonzero(is_dup)[0].tolist()
        for b in range(N_BAND):
            idxs = np.nonzero((~is_dup) & (uniq // BAND_ROWS == b))[0]
            take = idxs[:128]
            slot_of_uniq[take] = 128 * b + np.arange(len(take))
            overflow.extend(idxs[128:].tolist())
        overflow = np.array(sorted(overflow), dtype=np.int64)
        if len(overflow) > OV_CAP:
            raise ValueError(f"core {k}: overflow {len(overflow)} > {OV_CAP}")
        slot_of_uniq[overflow] = U_BAND + np.arange(len(overflow))

        # occurrence rank of each pair within its class (batch order preserved)
        o = np.argsort(inv, kind="stable")
        grp = inv[o]
        starts = np.r_[True, grp[1:] != grp[:-1]]
        pos_in_sorted = np.arange(len(inv))
        run_start = np.maximum.accumulate(np.where(starts, pos_in_sorted, 0))
        occ = np.empty(len(inv), dtype=np.int64)
        occ[o] = pos_in_sorted - run_start

        slot = slot_of_uniq[inv]
        if not (slot[occ >= 1] >= U_BAND).all():
            raise ValueError(f"core {k}: duplicate class outside overflow")
        gridpos = np.where(
            occ == 0, slot,
            np.where(occ == 1,
                     (N_UT * 128) + (slot - U_BAND),
                     ((N_UT + 1) * 128) + (slot - U_BAND)),
        )

        shard = centers_np[base:base + CPC]       # contiguous view, no copy
        row0 = shard[0]

        fg = np.zeros((128, N_T, FEAT_DIM), dtype=np.float32)  # pads: 0
        fg[gridpos % 128, gridpos // 128] = feats_np[sel]
        fg = fg.reshape(128, N_T * FEAT_DIM)

        gi = np.full(GRID, CPC, dtype=np.int32)   # pads dropped (OOB)
        gi[gridpos] = loc.astype(np.int32)

        si = np.full(N_SLOTS, CPC, dtype=np.int32)  # pads scatter out of bounds
        si[slot_of_uniq] = uniq.astype(np.int32)

        ns = np.zeros(N_SLOTS, dtype=np.float32)
        ns[slot_of_uniq] = -(
            np.float32(ALPHA) / (cnt.astype(np.float32) + np.float32(EPS))
        )

        meta = np.concatenate(
            [gi.reshape(N_T, 128).T, si.reshape(N_UT, 128).T], axis=1
        )
        in_maps.append({
            "cin": shard,
            "feats": fg,
            "meta": np.ascontiguousarray(meta),
            "nscale": np.ascontiguousarray(ns.reshape(N_UT, 128).T),
        })
    return in_maps


def kernel(features, target, centers):
    global _PROGRAM
    if _PROGRAM is None:
        _PROGRAM = _build_program()

    in_maps = _route(features, target, centers)
    res = bass_utils.run_bass_kernel_spmd(
        _PROGRAM, in_maps, core_ids=list(range(M))
    ).results

    new_centers = np.empty((NUM_CLASSES, FEAT_DIM), dtype=np.float32)
    loss_sum = 0.0
    for k in range(M):
        new_centers[k * CPC:(k + 1) * CPC] = res[k]["cout"]
        loss_sum += float(np.sum(res[k]["lossv"], dtype=np.float64))

    center_loss = np.float32(loss_sum / (BATCH * FEAT_DIM))
    return center_loss, new_centers
